# revision 4
# baseline (speedup 1.0000x reference)
"""Trainium2 Bass kernel for nn_AttentionModule (B=8, C=256, L=2048, D=32).

Per-batch computation (data-parallel: one batch per NeuronCore, 8 cores):
    qT = Wq @ x + bq            # (D, L)
    kT = Wk @ x + bk            # (D, L)
    vT = x.T @ (g*Wv).T + g*bv  # (L, C)   -- gamma folded into v
    ST = kT.T @ qT              # (L_j, L_i) = S[i,j] transposed
    E  = exp(ST)                # no max-subtraction: max|S| ~ 46, exp fits fp32
    Z  = sum_j E[j, i]          # DVE accumulate + PE ones-row reduce
    U  = vT.T @ E               # (C, L_i)  (already gamma-scaled)
    y  = U / Z + x

v2 design (from trace analysis of the 97us v1):
  - The 1/Z transpose+broadcast ran fp32 LOW_HIGH matmuls (~30us of PE).
    Replaced by: z_row[1,512] = ones.T @ zacc (2 bf16 MMs), reciprocal on
    the row, rb[128,512] = onesr.T x rrow outer-product (1 bf16 MM).
  - Software-pipelined emission: scores(t)/exp(t) are emitted ahead of
    U(t-1), so ACT (the bottleneck: 32 exps x 1.15us) never waits on
    tail work. vT projection is interleaved under the first two exps.
  - q/k bias copyback moved from ACT to DVE (tensor_scalar_add).
  - ACT exp-table prefetched via a dummy activation at t=0; PE prewarmed
    with junk matmuls during the input DMA wait (HAM clock-gate).
  - DMA triggers ordered: compute inputs first, fp32 x (residual-only)
    after the attention loop starts; y written per-quarter.
"""

import numpy as np

B, C, L, D = 8, 256, 2048, 32
NCORES = 8

_cache = {}


def _build_nc():
    from contextlib import ExitStack

    import concourse.bacc as bacc
    import concourse.tile as tile
    from concourse import mybir

    f32 = mybir.dt.float32
    bf16 = mybir.dt.bfloat16
    EXP = mybir.ActivationFunctionType.Exp

    nc = bacc.Bacc("TRN2", target_bir_lowering=False, debug=False)

    x_d = nc.dram_tensor("x", [C, L], f32, kind="ExternalInput")
    xb_d = nc.dram_tensor("xb", [C, L], bf16, kind="ExternalInput")
    wqk_d = nc.dram_tensor("wqk", [C, 2 * D], bf16, kind="ExternalInput")
    wvT_d = nc.dram_tensor("wvT", [C, C], bf16, kind="ExternalInput")
    bqk_d = nc.dram_tensor("bqk", [2 * D, 1], f32, kind="ExternalInput")
    bvr_d = nc.dram_tensor("bvr", [128, 4 * C], f32, kind="ExternalInput")
    y_d = nc.dram_tensor("y", [C, L], f32, kind="ExternalOutput")

    x_ap = x_d.ap()
    y_ap = y_d.ap()

    with tile.TileContext(nc) as tc, ExitStack() as ctx:
        singles = ctx.enter_context(tc.tile_pool(name="singles", bufs=1))
        big = ctx.enter_context(tc.tile_pool(name="big", bufs=1))
        ps = ctx.enter_context(tc.tile_pool(name="ps", bufs=2, space="PSUM"))
        up = ctx.enter_context(tc.tile_pool(name="up", bufs=1, space="PSUM"))
        zp = ctx.enter_context(tc.tile_pool(name="zp", bufs=1, space="PSUM"))
        epool = ctx.enter_context(tc.tile_pool(name="epool", bufs=6))
        ypool = ctx.enter_context(tc.tile_pool(name="ypool", bufs=4))
        uspool = ctx.enter_context(tc.tile_pool(name="uspool", bufs=2))
        rpool = ctx.enter_context(tc.tile_pool(name="rpool", bufs=2))

        # ---- on-chip constants (no DMA) ----
        ones_sb = singles.tile([128, 1], bf16, tag="ones")
        nc.vector.memset(ones_sb[:], 1.0)
        onesr_sb = singles.tile([1, 128], bf16, tag="onesr")
        nc.vector.memset(onesr_sb[:], 1.0)
        dummy_sb = singles.tile([1, 2], f32, tag="dummy")
        nc.vector.memset(dummy_sb[:], 0.0)
        # prefetch the exp table set while input DMAs run
        nc.scalar.activation(dummy_sb[0:1, 1:2], dummy_sb[0:1, 0:1], EXP)

        # ---- input DMA triggers: compute inputs first ----
        xb_sb = []
        for ct in range(2):
            tb = big.tile([128, L], bf16, tag=f"xb{ct}")
            nc.sync.dma_start(out=tb[:], in_=xb_d.ap()[ct * 128:(ct + 1) * 128, :])
            xb_sb.append(tb)
        wqk_sb = []
        for ct in range(2):
            tq = singles.tile([128, 2 * D], bf16, tag=f"wqk{ct}")
            nc.sync.dma_start(out=tq[:], in_=wqk_d.ap()[ct * 128:(ct + 1) * 128, :])
            wqk_sb.append(tq)
        bqk_sb = singles.tile([2 * D, 1], f32, tag="bqk")
        nc.sync.dma_start(out=bqk_sb[:], in_=bqk_d.ap()[:, :])
        wvT_sb = []
        for ct in range(2):
            tv = singles.tile([128, C], bf16, tag=f"wv{ct}")
            nc.sync.dma_start(out=tv[:], in_=wvT_d.ap()[ct * 128:(ct + 1) * 128, :])
            wvT_sb.append(tv)
        bvr_sb = singles.tile([128, 4 * C], f32, tag="bvr")
        nc.sync.dma_start(out=bvr_sb[:], in_=bvr_d.ap()[:, :])

        # ---- PE prewarm: junk matmuls so HAM un-throttles before qk ----
        rbwarm = zp.tile([128, 512], f32, tag="rb", name="rbwarm")
        for w in range(24):
            nc.tensor.matmul(
                rbwarm[:, 0:128], lhsT=onesr_sb[:], rhs=onesr_sb[:],
                start=True, stop=True,
            )

        # ---- q/k projection ----
        # qkT: q at partitions 0-31 (doubles as score rhs for strip g=0),
        # k at partitions 32-63.
        qkT = big.tile([64, L], bf16, tag="qkT")
        for it in range(4):
            p = ps.tile([128, 1024], f32, tag="ps")
            for ct in range(2):
                nc.tensor.matmul(
                    p[:2 * D, 0:512],
                    lhsT=wqk_sb[ct][:],
                    rhs=xb_sb[ct][:, it * 512:(it + 1) * 512],
                    start=(ct == 0),
                    stop=(ct == 1),
                )
            nc.vector.tensor_scalar_add(
                qkT[0:64, it * 512:(it + 1) * 512], p[0:64, 0:512],
                bqk_sb[0:64, 0:1],
            )

        # kT4: strip g holds kT j-blocks {4J+g}; round J lives at free cols J*128.
        kT4 = big.tile([128, 512], bf16, tag="kT4")
        for g in range(4):
            nc.sync.dma_start(
                out=kT4[32 * g:32 * (g + 1), :].rearrange("d (J j) -> d J j", j=128),
                in_=qkT[32:64, :].rearrange("d (J G j) -> d J G j", G=4, j=128)[:, :, g, :],
            )
        # qT4r: q replicated to partition strips 1..3 (strip 0 comes from qkT)
        qT4r = big.tile([128, L], bf16, tag="qT4r")
        for g in range(1, 4):
            nc.sync.dma_start(out=qT4r[32 * g:32 * (g + 1), :], in_=qkT[0:32, :])

        # fp32 x (residual only; needed first at quarter-0 finalize)
        x_sb = []
        for ct in range(2):
            t = big.tile([128, L], f32, tag=f"x{ct}")
            nc.sync.dma_start(out=t[:], in_=x_ap[ct * 128:(ct + 1) * 128, :])
            x_sb.append(t)

        # vT[j, c] as [128, 16*256]: block jb holds vT[jb*128 + p, c].
        vT_sb = big.tile([128, 16 * C], bf16, tag="vT")

        def emit_vt_group(grp):
            p = ps.tile([128, 1024], f32, tag="ps")
            for lbr in range(4):
                lb = 4 * grp + lbr
                for ct in range(2):
                    nc.tensor.matmul(
                        p[:, lbr * C:(lbr + 1) * C],
                        lhsT=xb_sb[ct][:, lb * 128:(lb + 1) * 128],
                        rhs=wvT_sb[ct][:],
                        start=(ct == 0),
                        stop=(ct == 1),
                    )
            nc.vector.tensor_add(
                vT_sb[:, grp * 1024:(grp + 1) * 1024], p[:, :], bvr_sb[:]
            )

        # ---- attention pipeline ----
        state = {}

        def emit_scores(t):
            qd, J = divmod(t, 4)
            i0 = qd * 512
            e_tiles = []
            for pair in range(2):
                stp = ps.tile([128, 1024], f32, tag="ps")
                for h in range(2):
                    g = 2 * pair + h
                    qsrc = qkT if g == 0 else qT4r
                    nc.tensor.matmul(
                        stp[:, h * 512:(h + 1) * 512],
                        lhsT=kT4[32 * g:32 * (g + 1), J * 128:(J + 1) * 128],
                        rhs=qsrc[32 * g:32 * (g + 1), i0:i0 + 512],
                        start=True,
                        stop=True,
                        tile_position=(32 * g, 0),
                    )
                e2 = epool.tile([128, 1024], bf16, tag="e", name="e2")
                nc.scalar.activation(e2[:], stp[:], EXP)
                e_tiles.append(e2)
            state[t] = e_tiles

        def emit_u_zacc(u):
            qd, J = divmod(u, 4)
            if J == 0:
                state[f"u{qd}"] = [
                    up.tile([128, 512], f32, tag=f"u{ct}", name=f"u{ct}", bufs=1)
                    for ct in range(2)
                ]
                state[f"zA{qd}"] = rpool.tile([128, 512], bf16, tag="zaccA", name="zaccA")
                state[f"zB{qd}"] = rpool.tile([128, 512], bf16, tag="zaccB", name="zaccB")
            u_t = state[f"u{qd}"]
            e_tiles = state.pop(u)
            for g in range(4):
                jb = 4 * J + g
                eh = e_tiles[g // 2][:, (g % 2) * 512:(g % 2 + 1) * 512]
                for ct in range(2):
                    nc.tensor.matmul(
                        u_t[ct][:, :],
                        lhsT=vT_sb[:, jb * C + ct * 128:jb * C + ct * 128 + 128],
                        rhs=eh,
                        start=(jb == 0),
                        stop=(jb == 15),
                    )
                ztgt = state[f"zA{qd}"] if jb % 2 == 0 else state[f"zB{qd}"]
                if jb in (0, 1):
                    nc.vector.tensor_copy(ztgt[:], eh)
                else:
                    nc.vector.tensor_add(ztgt[:], ztgt[:], eh)

        def emit_ucopy(qd):
            us = []
            for ct in range(2):
                u = uspool.tile([128, 512], f32, tag=f"us{ct}", name=f"us{ct}")
                nc.vector.tensor_copy(u[:], state[f"u{qd}"][ct][:, :])
                us.append(u)
            state[f"us{qd}"] = us

        def emit_zrow_recip(qd):
            zrow = zp.tile([1, 512], f32, tag="zrow", name="zrow")
            nc.tensor.matmul(
                zrow[0:1, :], lhsT=ones_sb[:], rhs=state[f"zA{qd}"][:],
                start=True, stop=False,
            )
            nc.tensor.matmul(
                zrow[0:1, :], lhsT=ones_sb[:], rhs=state[f"zB{qd}"][:],
                start=False, stop=True,
            )
            rrow = rpool.tile([1, 512], bf16, tag="rrow", name="rrow")
            with nc.allow_low_precision(reason="1/Z in bf16: 0.4% rel on the attn term, well under tolerance"):
                nc.vector.reciprocal(rrow[:], zrow[0:1, :])
            state[f"rr{qd}"] = rrow

        def emit_rb(qd, to_sbuf=True):
            rb_ps = zp.tile([128, 512], f32, tag="rb", name="rb_ps")
            nc.tensor.matmul(
                rb_ps[:, :], lhsT=onesr_sb[:], rhs=state[f"rr{qd}"][0:1, :],
                start=True, stop=True,
            )
            rb_sb = rpool.tile([128, 512], f32, tag="rb_sb", name="rb_sb")
            nc.vector.tensor_copy(rb_sb[:], rb_ps[:, :])
            state[f"rb{qd}"] = rb_sb

        def emit_finalize(qd, last=False):
            i0 = qd * 512
            eng = nc.vector if last else nc.gpsimd
            src = state[f"u{qd}"] if last else state[f"us{qd}"]
            for ct in range(2):
                yt = ypool.tile([128, 512], f32, tag="y", name="yt")
                eng.tensor_mul(yt[:], src[ct][:, :], state[f"rb{qd}"][:])
                eng.tensor_add(yt[:], yt[:], x_sb[ct][:, i0:i0 + 512])
                nc.sync.dma_start(
                    out=y_ap[ct * 128:(ct + 1) * 128, i0:i0 + 512], in_=yt[:]
                )

        for t in range(17):
            if t <= 15:
                emit_scores(t)
            if t == 0:
                emit_vt_group(0)
                emit_vt_group(1)
            elif t == 1:
                emit_vt_group(2)
                emit_vt_group(3)
            u = t - 1
            if u < 0:
                continue
            qd, J = divmod(u, 4)
            if u == 15:
                # last quarter: z-chain first, finalize straight from PSUM
                emit_u_zacc(u)
                emit_zrow_recip(3)
                emit_rb(3)
                emit_finalize(3, last=True)
                continue
            if J == 0 and qd > 0:
                emit_ucopy(qd - 1)          # DVE: free u psum for this U round
            emit_u_zacc(u)
            if J == 0 and qd > 0:
                emit_zrow_recip(qd - 1)     # PE z-reduce + DVE recip
            elif J == 1 and qd > 0:
                emit_rb(qd - 1)             # PE broadcast + DVE copy
            elif J == 2 and qd > 0:
                emit_finalize(qd - 1)       # GPSIMD + y DMA

    nc.compile()
    return nc


def get_nc():
    if "nc" not in _cache:
        _cache["nc"] = _build_nc()
    return _cache["nc"]


def make_in_maps(x, Wq, bq, Wk, bk, Wv, bv, gamma):
    import ml_dtypes

    bf = ml_dtypes.bfloat16
    x = np.asarray(x, dtype=np.float32)
    g = float(np.asarray(gamma, np.float32).reshape(-1)[0])
    gbv = (g * np.asarray(bv, np.float32)).reshape(1, C)
    shared = {
        "wqk": np.ascontiguousarray(
            np.concatenate([np.asarray(Wq, np.float32).T,
                            np.asarray(Wk, np.float32).T], axis=1)).astype(bf),
        "wvT": np.ascontiguousarray(g * np.asarray(Wv, np.float32).T).astype(bf),
        "bqk": np.concatenate([np.asarray(bq, np.float32).reshape(D, 1),
                               np.asarray(bk, np.float32).reshape(D, 1)], axis=0),
        "bvr": np.broadcast_to(np.tile(gbv, (1, 4)), (128, 4 * C)).copy(),
    }
    return [
        dict(shared, x=np.ascontiguousarray(x[b]), xb=np.ascontiguousarray(x[b]).astype(bf))
        for b in range(B)
    ]


def kernel(x, Wq, bq, Wk, bk, Wv, bv, gamma):
    from concourse.bass_utils import run_bass_kernel_spmd

    nc = get_nc()
    in_maps = make_in_maps(x, Wq, bq, Wk, bk, Wv, bv, gamma)
    res = run_bass_kernel_spmd(nc, in_maps, list(range(NCORES)))
    return np.stack([res.results[b]["y"] for b in range(B)], axis=0)


# revision 8
# speedup vs baseline: 1.0840x; 1.0840x over previous
"""Trainium2 Bass kernel for nn_AttentionModule (B=8, C=256, L=2048, D=32).

Per-batch computation (data-parallel: one batch per NeuronCore, 8 cores):
    qT = Wq @ x + bq            # (D, L)
    kT = Wk @ x + bk            # (D, L)
    vT = x.T @ (g*Wv).T + g*bv  # (L, C)   -- gamma folded into v
    ST = kT.T @ qT              # (L_j, L_i) = S[i,j] transposed
    E  = exp(ST)                # no max-subtraction: max|S| ~ 46, exp fits fp32
    Z  = sum_j E[j, i]          # DVE accumulate + PE ones-reduce
    U  = vT.T @ E               # (C, L_i)  (already gamma-scaled)
    y  = U / Z + x

v3 design notes (from v1/v2 trace analysis):
  - ACT is the floor: 32 exps x ~1.15us. Emission is software-pipelined so
    ACT never waits: scores(t)/exp(t) lead U(t-1); vT is produced under the
    first two exp rounds; all per-quarter tail work trails by 1-3 rounds.
  - GPSIMD is NOT used: its SBUF port is shared with DVE and measured DVE
    throughput drops 3.5x while GPSIMD runs (v2: z-acc TT 500 -> 1700ns).
  - DVE reciprocal is ~8 cycles/element/lane, so 1/Z runs on the [128,4]
    zt layout (~170ns), not on a [1,512] row (4us, v2 mistake). The
    transpose back to a row + broadcast to [128,512] are bf16 matmuls
    (v1 ran these in fp32 LOW_HIGH: ~30us of PE).
  - Z accumulate: one TT per [128,1024] e-tile into a combined A|B
    accumulator (halves DVE op count).
  - v bias folded in as a PE outer-product (ones x bv) accumulated into
    the projection psum; copybacks are plain copies split ACT/DVE.
  - exp table prefetched via dummy activation; PE prewarmed with junk
    matmuls during the input DMA wait (HAM clock gate).
  - Small SBUF tiles padded to 64B/partition so big tiles stay aligned.
"""

import numpy as np

B, C, L, D = 8, 256, 2048, 32
NCORES = 8

_cache = {}


def _build_nc():
    from contextlib import ExitStack

    import concourse.bacc as bacc
    import concourse.tile as tile
    from concourse import mybir

    f32 = mybir.dt.float32
    bf16 = mybir.dt.bfloat16
    EXP = mybir.ActivationFunctionType.Exp
    IDENT = mybir.ActivationFunctionType.Identity

    nc = bacc.Bacc("TRN2", target_bir_lowering=False, debug=False)

    x_d = nc.dram_tensor("x", [C, L], f32, kind="ExternalInput")
    xb_d = nc.dram_tensor("xb", [C, L], bf16, kind="ExternalInput")
    wqk_d = nc.dram_tensor("wqk", [C, 2 * D], bf16, kind="ExternalInput")
    wvT_d = nc.dram_tensor("wvT", [C, C], bf16, kind="ExternalInput")
    bqk_d = nc.dram_tensor("bqk", [2 * D, 1], f32, kind="ExternalInput")
    bvb4_d = nc.dram_tensor("bvb4", [1, 4 * C], bf16, kind="ExternalInput")
    identb_d = nc.dram_tensor("identb", [128, 128], bf16, kind="ExternalInput")
    y_d = nc.dram_tensor("y", [C, L], f32, kind="ExternalOutput")

    x_ap = x_d.ap()
    y_ap = y_d.ap()

    with tile.TileContext(nc) as tc, ExitStack() as ctx:
        singles = ctx.enter_context(tc.tile_pool(name="singles", bufs=1))
        big = ctx.enter_context(tc.tile_pool(name="big", bufs=1))
        ps = ctx.enter_context(tc.tile_pool(name="ps", bufs=2, space="PSUM"))
        up = ctx.enter_context(tc.tile_pool(name="up", bufs=1, space="PSUM"))
        epool = ctx.enter_context(tc.tile_pool(name="epool", bufs=6))
        ypool = ctx.enter_context(tc.tile_pool(name="ypool", bufs=4))
        uspool = ctx.enter_context(tc.tile_pool(name="uspool", bufs=2))
        rpool = ctx.enter_context(tc.tile_pool(name="rpool", bufs=2))

        # ---- on-chip constants (padded to 64B/partition for alignment) ----
        ones_sb = singles.tile([128, 32], bf16, tag="ones")
        nc.vector.memset(ones_sb[:], 1.0)
        onesr_sb = singles.tile([1, 128], bf16, tag="onesr")
        nc.vector.memset(onesr_sb[:], 1.0)
        dummy_sb = singles.tile([1, 16], f32, tag="dummy")
        nc.vector.memset(dummy_sb[:], 0.0)
        # prefetch the exp table set while input DMAs run
        nc.scalar.activation(dummy_sb[0:1, 1:2], dummy_sb[0:1, 0:1], EXP)

        # ---- input DMA triggers: compute inputs first, fp32 x later ----
        xb_sb = []
        for ct in range(2):
            tb = big.tile([128, L], bf16, tag=f"xb{ct}")
            nc.sync.dma_start(out=tb[:], in_=xb_d.ap()[ct * 128:(ct + 1) * 128, :])
            xb_sb.append(tb)
        wqk_sb = []
        for ct in range(2):
            tq = singles.tile([128, 2 * D], bf16, tag=f"wqk{ct}")
            nc.sync.dma_start(out=tq[:], in_=wqk_d.ap()[ct * 128:(ct + 1) * 128, :])
            wqk_sb.append(tq)
        bqk_sb = singles.tile([2 * D, 16], f32, tag="bqk")
        nc.sync.dma_start(out=bqk_sb[:, 0:1], in_=bqk_d.ap()[:, :])
        wvT_sb = []
        for ct in range(2):
            tv = singles.tile([128, C], bf16, tag=f"wv{ct}")
            nc.sync.dma_start(out=tv[:], in_=wvT_d.ap()[ct * 128:(ct + 1) * 128, :])
            wvT_sb.append(tv)
        bvb4_sb = singles.tile([1, 4 * C], bf16, tag="bvb4")
        nc.sync.dma_start(out=bvb4_sb[:], in_=bvb4_d.ap()[:, :])
        identb_sb = singles.tile([128, 128], bf16, tag="identb")
        nc.sync.dma_start(out=identb_sb[:], in_=identb_d.ap()[:, :])

        # ---- PE prewarm: junk matmuls so HAM un-throttles before qk ----
        rbwarm = ps.tile([128, 512], f32, tag="rb", bufs=1, name="rbwarm")
        for w in range(24):
            nc.tensor.matmul(
                rbwarm[:, 0:128], lhsT=onesr_sb[:], rhs=onesr_sb[:],
                start=True, stop=True,
            )

        # ---- q/k projection; bias via ACT copyback (ACT is idle pre-loop) ----
        # qkT: q at partitions 0-31 (doubles as score rhs for strip g=0),
        # k at partitions 32-63.
        qkT = big.tile([64, L], bf16, tag="qkT")
        for it in range(4):
            p = ps.tile([128, 1024], f32, tag="ps")
            for ct in range(2):
                nc.tensor.matmul(
                    p[:2 * D, 0:512],
                    lhsT=wqk_sb[ct][:],
                    rhs=xb_sb[ct][:, it * 512:(it + 1) * 512],
                    start=(ct == 0),
                    stop=(ct == 1),
                )
            nc.scalar.activation(
                qkT[0:64, it * 512:(it + 1) * 512], p[0:64, 0:512], IDENT,
                bias=bqk_sb[0:64, 0:1],
            )

        # kT4: strip g holds kT j-blocks {4J+g}; round J lives at free cols J*128.
        kT4 = big.tile([128, 512], bf16, tag="kT4")
        for g in range(4):
            nc.sync.dma_start(
                out=kT4[32 * g:32 * (g + 1), :].rearrange("d (J j) -> d J j", j=128),
                in_=qkT[32:64, :].rearrange("d (J G j) -> d J G j", G=4, j=128)[:, :, g, :],
            )
        # qT4r: q replicated to partition strips 1..3 (strip 0 comes from qkT)
        qT4r = big.tile([128, L], bf16, tag="qT4r")
        for g in range(1, 4):
            nc.sync.dma_start(out=qT4r[32 * g:32 * (g + 1), :], in_=qkT[0:32, :])

        # fp32 x (residual only; first needed at quarter-0 finalize)
        x_sb = []
        for ct in range(2):
            t = big.tile([128, L], f32, tag=f"x{ct}")
            nc.sync.dma_start(out=t[:], in_=x_ap[ct * 128:(ct + 1) * 128, :])
            x_sb.append(t)

        # vT[j, c] as [128, 16*256]: block jb holds vT[jb*128 + p, c].
        vT_sb = big.tile([128, 16 * C], bf16, tag="vT")

        def emit_vt_group(grp, eng):
            p = ps.tile([128, 1024], f32, tag="ps")
            for hb in range(2):
                # bias first: zeroes the half-bank with ones x (g*bv tiled)
                nc.tensor.matmul(
                    p[:, hb * 512:(hb + 1) * 512],
                    lhsT=onesr_sb[:], rhs=bvb4_sb[0:1, hb * 512:(hb + 1) * 512],
                    start=True, stop=False, skip_group_check=True,
                )
                for lbr in (2 * hb, 2 * hb + 1):
                    lb = 4 * grp + lbr
                    for ct in range(2):
                        nc.tensor.matmul(
                            p[:, lbr * C:(lbr + 1) * C],
                            lhsT=xb_sb[ct][:, lb * 128:(lb + 1) * 128],
                            rhs=wvT_sb[ct][:],
                            start=False,
                            stop=(lbr == 2 * hb + 1 and ct == 1),
                            skip_group_check=True,
                        )
            dst = vT_sb[:, grp * 1024:(grp + 1) * 1024]
            if eng == "act":
                nc.scalar.copy(dst, p[:, :])
            else:
                nc.vector.tensor_copy(dst, p[:, :])

        # ---- attention pipeline ----
        state = {}

        def emit_scores(t):
            qd, J = divmod(t, 4)
            i0 = qd * 512
            e_tiles = []
            for pair in range(2):
                stp = ps.tile([128, 1024], f32, tag="ps")
                for h in range(2):
                    g = 2 * pair + h
                    qsrc = qkT if g == 0 else qT4r
                    nc.tensor.matmul(
                        stp[:, h * 512:(h + 1) * 512],
                        lhsT=kT4[32 * g:32 * (g + 1), J * 128:(J + 1) * 128],
                        rhs=qsrc[32 * g:32 * (g + 1), i0:i0 + 512],
                        start=True,
                        stop=True,
                        tile_position=(32 * g, 0),
                    )
                e2 = epool.tile([128, 1024], bf16, tag="e", name="e2")
                nc.scalar.activation(e2[:], stp[:], EXP)
                e_tiles.append(e2)
            state[t] = e_tiles

        def emit_u_zacc(u):
            qd, J = divmod(u, 4)
            if J == 0:
                state[f"u{qd}"] = [
                    up.tile([128, 512], f32, tag=f"u{ct}", name=f"u{ct}", bufs=1)
                    for ct in range(2)
                ]
                state[f"z{qd}"] = rpool.tile([128, 1024], bf16, tag="zacc", name="zacc")
            u_t = state[f"u{qd}"]
            e_tiles = state.pop(u)
            for g in range(4):
                jb = 4 * J + g
                eh = e_tiles[g // 2][:, (g % 2) * 512:(g % 2 + 1) * 512]
                for ct in range(2):
                    nc.tensor.matmul(
                        u_t[ct][:, :],
                        lhsT=vT_sb[:, jb * C + ct * 128:jb * C + ct * 128 + 128],
                        rhs=eh,
                        start=(jb == 0),
                        stop=(jb == 15),
                    )
            zacc = state[f"z{qd}"]
            for pair in range(2):
                if J == 0 and pair == 0:
                    nc.vector.tensor_copy(zacc[:], e_tiles[0][:])
                elif J == 0 and pair == 1:
                    # fold pair1 in: A|B layout keeps one TT per e-tile
                    nc.vector.tensor_add(zacc[:], zacc[:], e_tiles[1][:])
                else:
                    nc.vector.tensor_add(zacc[:], zacc[:], e_tiles[pair][:])

        def emit_ucopy(qd):
            us = []
            for ct in range(2):
                u = uspool.tile([128, 512], f32, tag=f"us{ct}", name=f"us{ct}")
                nc.vector.tensor_copy(u[:], state[f"u{qd}"][ct][:, :])
                us.append(u)
            state[f"us{qd}"] = us

        def emit_zt_recip(qd):
            zacc = state[f"z{qd}"]
            zab = rpool.tile([128, 512], bf16, tag="zab", name="zab")
            nc.vector.tensor_add(zab[:], zacc[:, 0:512], zacc[:, 512:1024])
            zt = ps.tile([128, 512], f32, tag="zmisc", bufs=1, name="zt")
            for c in range(4):
                nc.tensor.matmul(
                    zt[:, c:c + 1],
                    lhsT=zab[:, 128 * c:128 * (c + 1)],
                    rhs=ones_sb[:, 0:1],
                    start=True,
                    stop=True,
                )
            rt = rpool.tile([128, 32], bf16, tag="rt", name="rt")
            with nc.allow_low_precision(reason="1/Z in bf16: 0.4% rel on the attn term, well under tolerance"):
                nc.vector.reciprocal(rt[:, 0:4], zt[:, 0:4])
            state[f"rt{qd}"] = rt

        def emit_rb(qd):
            rt = state[f"rt{qd}"]
            rd_ps = ps.tile([128, 512], f32, tag="zmisc", bufs=1, name="rd_ps")
            for c in range(4):
                nc.tensor.matmul(
                    rd_ps[0:1, 128 * c:128 * (c + 1)],
                    lhsT=rt[:, c:c + 1],
                    rhs=identb_sb[:],
                    start=True,
                    stop=True,
                )
            rd = rpool.tile([1, 512], bf16, tag="rd", name="rd")
            nc.vector.tensor_copy(rd[:], rd_ps[0:1, 0:512])
            rb_ps = ps.tile([128, 512], f32, tag="rb", bufs=1, name="rb_ps")
            nc.tensor.matmul(
                rb_ps[:, :], lhsT=onesr_sb[:], rhs=rd[0:1, :],
                start=True, stop=True,
            )
            state[f"rb{qd}"] = rb_ps

        def emit_finalize(qd, last=False):
            i0 = qd * 512
            if last:
                # read u straight from PSUM; rb must come from SBUF instead
                rb_sb = rpool.tile([128, 512], f32, tag="rb_sb", name="rb_sb")
                nc.vector.tensor_copy(rb_sb[:], state[f"rb{qd}"][:, :])
                src = state[f"u{qd}"]
                rb = rb_sb
            else:
                src = state[f"us{qd}"]
                rb = state[f"rb{qd}"]
            for ct in range(2):
                yt = ypool.tile([128, 512], f32, tag="y", name="yt")
                nc.vector.tensor_mul(yt[:], src[ct][:, :], rb[:, 0:512] if last else rb[:, :])
                nc.vector.tensor_add(yt[:], yt[:], x_sb[ct][:, i0:i0 + 512])
                nc.sync.dma_start(
                    out=y_ap[ct * 128:(ct + 1) * 128, i0:i0 + 512], in_=yt[:]
                )

        for t in range(17):
            if t <= 15:
                emit_scores(t)
            if t == 0:
                emit_vt_group(0, "act")
                emit_vt_group(1, "dve")
            elif t == 1:
                emit_vt_group(2, "act")
                emit_vt_group(3, "dve")
            u = t - 1
            if u < 0:
                continue
            qd, J = divmod(u, 4)
            if u == 15:
                emit_u_zacc(u)
                emit_zt_recip(3)
                emit_rb(3)
                emit_finalize(3, last=True)
                continue
            if J == 0 and qd > 0:
                emit_ucopy(qd - 1)          # DVE: free u psum for this U round
            emit_u_zacc(u)
            if qd > 0:
                if J == 0:
                    emit_zt_recip(qd - 1)
                elif J == 1:
                    emit_rb(qd - 1)
                elif J == 2:
                    emit_finalize(qd - 1)

    nc.compile()
    return nc


def get_nc():
    if "nc" not in _cache:
        _cache["nc"] = _build_nc()
    return _cache["nc"]


def make_in_maps(x, Wq, bq, Wk, bk, Wv, bv, gamma):
    import ml_dtypes

    bf = ml_dtypes.bfloat16
    x = np.asarray(x, dtype=np.float32)
    g = float(np.asarray(gamma, np.float32).reshape(-1)[0])
    gbv = (g * np.asarray(bv, np.float32)).reshape(1, C)
    shared = {
        "wqk": np.ascontiguousarray(
            np.concatenate([np.asarray(Wq, np.float32).T,
                            np.asarray(Wk, np.float32).T], axis=1)).astype(bf),
        "wvT": np.ascontiguousarray(g * np.asarray(Wv, np.float32).T).astype(bf),
        "bqk": np.concatenate([np.asarray(bq, np.float32).reshape(D, 1),
                               np.asarray(bk, np.float32).reshape(D, 1)], axis=0),
        "bvb4": np.ascontiguousarray(np.tile(gbv, (1, 4))).astype(bf),
        "identb": np.eye(128, dtype=np.float32).astype(bf),
    }
    return [
        dict(shared, x=np.ascontiguousarray(x[b]), xb=np.ascontiguousarray(x[b]).astype(bf))
        for b in range(B)
    ]


def kernel(x, Wq, bq, Wk, bk, Wv, bv, gamma):
    from concourse.bass_utils import run_bass_kernel_spmd

    nc = get_nc()
    in_maps = make_in_maps(x, Wq, bq, Wk, bk, Wv, bv, gamma)
    res = run_bass_kernel_spmd(nc, in_maps, list(range(NCORES)))
    return np.stack([res.results[b]["y"] for b in range(B)], axis=0)


# revision 10
# speedup vs baseline: 1.3660x; 1.2602x over previous
"""Trainium2 Bass kernel for nn_AttentionModule (B=8, C=256, L=2048, D=32).

Per-batch computation (data-parallel: one batch per NeuronCore, 8 cores):
    qT = Wq @ x + bq            # (D, L)
    kT = Wk @ x + bk            # (D, L)
    vT = x.T @ (g*Wv).T + g*bv  # (L, C)   -- gamma folded into v
    ST = kT.T @ qT              # (L_j, L_i), row-packed 4x (D=32)
    E  = exp(ST)                # no max-subtraction: max|S| ~ 46, exp fits fp32
    Z  = sum_j E[j, i]          # DVE accumulate + PE ones-reduce
    U  = vT.T @ E               # (C, L_i)  (already gamma-scaled)
    y  = U / Z + x

v4 design notes (from v1-v3 trace analysis):
  - ACT exp floor: 32 x ~1.34us (P0 power throttle: all engines run ~20%
    below nominal once the pipeline is dense). Emission is software-
    pipelined: scores(t)/exp(t) lead U(t-1); tail work trails 1-2 rounds.
  - q/k projection uses column-tiled weights [Wq x4] / [Wk x4], so q and
    k come out of PSUM already replicated across the four 32-partition
    strips that the row-packed score matmuls need. No SBUF gather/replica
    DMAs on the critical path. q-copyback on ACT (bias via activation),
    k-copyback on DVE (tensor_scalar_add), per 512-col chunk.
  - GPSIMD is NOT used: its SBUF port steals ~3.5x DVE throughput.
  - 1/Z runs on the [128,4] layout (DVE reciprocal is ~8 cyc/elem/lane);
    transpose-to-row and broadcast are bf16 matmuls (identity / ones).
  - Z accumulate: one TT per [128,1024] e-tile into a combined A|B
    accumulator.
  - v bias folded in as a PE outer-product (ones x bv); vT copybacks are
    plain copies split ACT/DVE, emitted under the first two exp rounds.
  - exp table prefetched via dummy activation; PE prewarmed with junk
    matmuls through the input-DMA window (HAM clock gate).
  - DMA triggers cost ~0.7us each on their queue: spread across Sync and
    Scalar queues, xb split into column-halves so qk starts early, fp32 x
    (residual only) rides behind the compute inputs.
"""

import numpy as np

B, C, L, D = 8, 256, 2048, 32
NCORES = 8

_cache = {}


def _build_nc():
    from contextlib import ExitStack

    import concourse.bacc as bacc
    import concourse.tile as tile
    from concourse import mybir

    f32 = mybir.dt.float32
    bf16 = mybir.dt.bfloat16
    EXP = mybir.ActivationFunctionType.Exp
    IDENT = mybir.ActivationFunctionType.Identity

    nc = bacc.Bacc("TRN2", target_bir_lowering=False, debug=False)

    x_d = nc.dram_tensor("x", [C, L], f32, kind="ExternalInput")
    xb_d = nc.dram_tensor("xb", [C, L], bf16, kind="ExternalInput")
    wqk4_d = nc.dram_tensor("wqk4", [C, 256], bf16, kind="ExternalInput")
    wvT_d = nc.dram_tensor("wvT", [C, C], bf16, kind="ExternalInput")
    bqk4_d = nc.dram_tensor("bqk4", [128, 2], f32, kind="ExternalInput")
    bvb4_d = nc.dram_tensor("bvb4", [1, 4 * C], bf16, kind="ExternalInput")
    identb_d = nc.dram_tensor("identb", [128, 128], bf16, kind="ExternalInput")
    y_d = nc.dram_tensor("y", [C, L], f32, kind="ExternalOutput")

    x_ap = x_d.ap()
    y_ap = y_d.ap()

    with tile.TileContext(nc) as tc, ExitStack() as ctx:
        singles = ctx.enter_context(tc.tile_pool(name="singles", bufs=1))
        big = ctx.enter_context(tc.tile_pool(name="big", bufs=1))
        ps = ctx.enter_context(tc.tile_pool(name="ps", bufs=2, space="PSUM"))
        up = ctx.enter_context(tc.tile_pool(name="up", bufs=1, space="PSUM"))
        epool = ctx.enter_context(tc.tile_pool(name="epool", bufs=6))
        ypool = ctx.enter_context(tc.tile_pool(name="ypool", bufs=4))
        uspool = ctx.enter_context(tc.tile_pool(name="uspool", bufs=2))
        rpool = ctx.enter_context(tc.tile_pool(name="rpool", bufs=2))

        # ---- on-chip constants (padded to 64B/partition for alignment) ----
        ones_sb = singles.tile([128, 32], bf16, tag="ones")
        nc.vector.memset(ones_sb[:], 1.0)
        onesr_sb = singles.tile([1, 128], bf16, tag="onesr")
        nc.vector.memset(onesr_sb[:], 1.0)
        dummy_sb = singles.tile([1, 16], f32, tag="dummy")
        nc.vector.memset(dummy_sb[:], 0.0)
        # prefetch the exp table set while input DMAs run
        nc.scalar.activation(dummy_sb[0:1, 1:2], dummy_sb[0:1, 0:1], EXP)

        # ---- input DMA triggers ----
        # Scalar queue: small qk weights (needed first)
        wqk4_sb = []
        for ct in range(2):
            tq = singles.tile([128, 256], bf16, tag=f"wqk4{ct}")
            nc.scalar.dma_start(out=tq[:], in_=wqk4_d.ap()[ct * 128:(ct + 1) * 128, :])
            wqk4_sb.append(tq)
        bqk4_sb = singles.tile([128, 32], f32, tag="bqk4")
        nc.scalar.dma_start(out=bqk4_sb[:, 0:2], in_=bqk4_d.ap()[:, :])

        # Sync queue: xb in column-halves, then v weights, then fp32 x
        xb_sb = [big.tile([128, L], bf16, tag=f"xb{ct}", name=f"xb{ct}") for ct in range(2)]
        for half in range(2):
            for ct in range(2):
                nc.sync.dma_start(
                    out=xb_sb[ct][:, half * 1024:(half + 1) * 1024],
                    in_=xb_d.ap()[ct * 128:(ct + 1) * 128, half * 1024:(half + 1) * 1024],
                )
        wvT_sb = []
        for ct in range(2):
            tv = singles.tile([128, C], bf16, tag=f"wv{ct}")
            nc.sync.dma_start(out=tv[:], in_=wvT_d.ap()[ct * 128:(ct + 1) * 128, :])
            wvT_sb.append(tv)
        bvb4_sb = singles.tile([1, 4 * C], bf16, tag="bvb4")
        nc.sync.dma_start(out=bvb4_sb[:], in_=bvb4_d.ap()[:, :])
        x_sb = [big.tile([128, L], f32, tag=f"x{ct}", name=f"x{ct}") for ct in range(2)]
        for ct in range(2):
            nc.sync.dma_start(out=x_sb[ct][:], in_=x_ap[ct * 128:(ct + 1) * 128, :])
        identb_sb = singles.tile([128, 128], bf16, tag="identb")
        nc.sync.dma_start(out=identb_sb[:], in_=identb_d.ap()[:, :])

        # ---- PE prewarm: junk matmuls so HAM un-throttles before qk ----
        rbwarm = ps.tile([128, 512], f32, tag="rb", bufs=1, name="rbwarm")
        for w in range(28):
            nc.tensor.matmul(
                rbwarm[:, 0:128], lhsT=onesr_sb[:], rhs=onesr_sb[:],
                start=True, stop=True,
            )

        # ---- q/k projection with strip-replication baked into the weights --
        # psum chunk: cols 0-511 = q replicated to 4 strips, 512-1023 = k
        # where strip g holds kT j-block {4*it+g} (exactly the kT4 layout).
        qT4x = big.tile([128, L], bf16, tag="qT4x")
        kT4 = big.tile([128, 512], bf16, tag="kT4")
        for it in range(4):
            p = ps.tile([128, 1024], f32, tag="ps")
            for ct in range(2):
                nc.tensor.matmul(
                    p[:, 0:512],
                    lhsT=wqk4_sb[ct][:, 0:128],
                    rhs=xb_sb[ct][:, it * 512:(it + 1) * 512],
                    start=(ct == 0),
                    stop=(ct == 1),
                )
            for ct in range(2):
                nc.tensor.matmul(
                    p[:, 512:1024],
                    lhsT=wqk4_sb[ct][:, 128:256],
                    rhs=xb_sb[ct][:, it * 512:(it + 1) * 512],
                    start=(ct == 0),
                    stop=(ct == 1),
                )
            nc.scalar.activation(
                qT4x[:, it * 512:(it + 1) * 512], p[:, 0:512], IDENT,
                bias=bqk4_sb[:, 0:1],
            )
            for g in range(4):
                nc.vector.tensor_scalar_add(
                    kT4[32 * g:32 * (g + 1), it * 128:(it + 1) * 128],
                    p[32 * g:32 * (g + 1), 512 + g * 128:512 + (g + 1) * 128],
                    bqk4_sb[32 * g:32 * (g + 1), 1:2],
                )

        # vT[j, c] as [128, 16*256]: block jb holds vT[jb*128 + p, c].
        vT_sb = big.tile([128, 16 * C], bf16, tag="vT")

        def emit_vt_group(grp, eng):
            p = ps.tile([128, 1024], f32, tag="ps")
            for hb in range(2):
                # bias first: zeroes the half-bank with ones x (g*bv tiled)
                nc.tensor.matmul(
                    p[:, hb * 512:(hb + 1) * 512],
                    lhsT=onesr_sb[:], rhs=bvb4_sb[0:1, hb * 512:(hb + 1) * 512],
                    start=True, stop=False, skip_group_check=True,
                )
                for lbr in (2 * hb, 2 * hb + 1):
                    lb = 4 * grp + lbr
                    for ct in range(2):
                        nc.tensor.matmul(
                            p[:, lbr * C:(lbr + 1) * C],
                            lhsT=xb_sb[ct][:, lb * 128:(lb + 1) * 128],
                            rhs=wvT_sb[ct][:],
                            start=False,
                            stop=(lbr == 2 * hb + 1 and ct == 1),
                            skip_group_check=True,
                        )
            dst = vT_sb[:, grp * 1024:(grp + 1) * 1024]
            if eng == "act":
                nc.scalar.copy(dst, p[:, :])
            else:
                nc.vector.tensor_copy(dst, p[:, :])

        # ---- attention pipeline ----
        state = {}

        def emit_scores(t):
            qd, J = divmod(t, 4)
            i0 = qd * 512
            e_tiles = []
            for pair in range(2):
                stp = ps.tile([128, 1024], f32, tag="ps")
                for h in range(2):
                    g = 2 * pair + h
                    nc.tensor.matmul(
                        stp[:, h * 512:(h + 1) * 512],
                        lhsT=kT4[32 * g:32 * (g + 1), J * 128:(J + 1) * 128],
                        rhs=qT4x[32 * g:32 * (g + 1), i0:i0 + 512],
                        start=True,
                        stop=True,
                        tile_position=(32 * g, 0),
                    )
                e2 = epool.tile([128, 1024], bf16, tag="e", name="e2")
                nc.scalar.activation(e2[:], stp[:], EXP)
                e_tiles.append(e2)
            state[t] = e_tiles

        def emit_u_zacc(u):
            qd, J = divmod(u, 4)
            if J == 0:
                state[f"u{qd}"] = [
                    up.tile([128, 512], f32, tag=f"u{ct}", name=f"u{ct}", bufs=1)
                    for ct in range(2)
                ]
                state[f"z{qd}"] = rpool.tile([128, 1024], bf16, tag="zacc", name="zacc")
            u_t = state[f"u{qd}"]
            e_tiles = state.pop(u)
            for g in range(4):
                jb = 4 * J + g
                eh = e_tiles[g // 2][:, (g % 2) * 512:(g % 2 + 1) * 512]
                for ct in range(2):
                    nc.tensor.matmul(
                        u_t[ct][:, :],
                        lhsT=vT_sb[:, jb * C + ct * 128:jb * C + ct * 128 + 128],
                        rhs=eh,
                        start=(jb == 0),
                        stop=(jb == 15),
                    )
            zacc = state[f"z{qd}"]
            for pair in range(2):
                if J == 0 and pair == 0:
                    nc.vector.tensor_copy(zacc[:], e_tiles[0][:])
                else:
                    nc.vector.tensor_add(zacc[:], zacc[:], e_tiles[pair][:])

        def emit_ucopy(qd):
            us = []
            for ct in range(2):
                u = uspool.tile([128, 512], f32, tag=f"us{ct}", name=f"us{ct}")
                nc.vector.tensor_copy(u[:], state[f"u{qd}"][ct][:, :])
                us.append(u)
            state[f"us{qd}"] = us

        def emit_zt_recip(qd):
            zacc = state[f"z{qd}"]
            zab = rpool.tile([128, 512], bf16, tag="zab", name="zab")
            nc.vector.tensor_add(zab[:], zacc[:, 0:512], zacc[:, 512:1024])
            zt = ps.tile([128, 512], f32, tag="zmisc", bufs=1, name="zt")
            for c in range(4):
                nc.tensor.matmul(
                    zt[:, c:c + 1],
                    lhsT=zab[:, 128 * c:128 * (c + 1)],
                    rhs=ones_sb[:, 0:1],
                    start=True,
                    stop=True,
                )
            rt = rpool.tile([128, 32], bf16, tag="rt", name="rt")
            with nc.allow_low_precision(reason="1/Z in bf16: 0.4% rel on the attn term, well under tolerance"):
                nc.vector.reciprocal(rt[:, 0:4], zt[:, 0:4])
            state[f"rt{qd}"] = rt

        def emit_rb(qd):
            rt = state[f"rt{qd}"]
            rd_ps = ps.tile([128, 512], f32, tag="zmisc", bufs=1, name="rd_ps")
            for c in range(4):
                nc.tensor.matmul(
                    rd_ps[0:1, 128 * c:128 * (c + 1)],
                    lhsT=rt[:, c:c + 1],
                    rhs=identb_sb[:],
                    start=True,
                    stop=True,
                )
            rd = rpool.tile([1, 512], bf16, tag="rd", name="rd")
            nc.vector.tensor_copy(rd[:], rd_ps[0:1, 0:512])
            rb_ps = ps.tile([128, 512], f32, tag="rb", bufs=1, name="rb_ps")
            nc.tensor.matmul(
                rb_ps[:, :], lhsT=onesr_sb[:], rhs=rd[0:1, :],
                start=True, stop=True,
            )
            state[f"rb{qd}"] = rb_ps

        def emit_finalize(qd, last=False):
            i0 = qd * 512
            if last:
                # read u straight from PSUM; rb must come from SBUF instead
                rb_sb = rpool.tile([128, 512], f32, tag="rb_sb", name="rb_sb")
                nc.vector.tensor_copy(rb_sb[:], state[f"rb{qd}"][:, :])
                src = state[f"u{qd}"]
                rb = rb_sb
            else:
                src = state[f"us{qd}"]
                rb = state[f"rb{qd}"]
            for ct in range(2):
                yt = ypool.tile([128, 512], f32, tag="y", name="yt")
                nc.vector.tensor_mul(yt[:], src[ct][:, :], rb[:, 0:512])
                nc.vector.tensor_add(yt[:], yt[:], x_sb[ct][:, i0:i0 + 512])
                nc.sync.dma_start(
                    out=y_ap[ct * 128:(ct + 1) * 128, i0:i0 + 512], in_=yt[:]
                )

        for t in range(17):
            if t <= 15:
                emit_scores(t)
            if t == 0:
                emit_vt_group(0, "act")
                emit_vt_group(1, "dve")
            elif t == 1:
                emit_vt_group(2, "act")
                emit_vt_group(3, "dve")
            u = t - 1
            if u < 0:
                continue
            qd, J = divmod(u, 4)
            if J == 0 and qd > 0:
                emit_ucopy(qd - 1)          # DVE: free u psum for this U round
            emit_u_zacc(u)
            if J == 3:
                emit_zt_recip(qd)           # z-chain starts same round it closes
            elif J == 0 and qd > 0:
                emit_rb(qd - 1)
            elif J == 1 and qd > 0:
                emit_finalize(qd - 1)
            if u == 15:
                emit_rb(3)
                emit_finalize(3, last=True)

    nc.compile()
    return nc


def get_nc():
    if "nc" not in _cache:
        _cache["nc"] = _build_nc()
    return _cache["nc"]


def make_in_maps(x, Wq, bq, Wk, bk, Wv, bv, gamma):
    import ml_dtypes

    bf = ml_dtypes.bfloat16
    x = np.asarray(x, dtype=np.float32)
    g = float(np.asarray(gamma, np.float32).reshape(-1)[0])
    gbv = (g * np.asarray(bv, np.float32)).reshape(1, C)
    wq = np.asarray(Wq, np.float32).T        # (C, D)
    wk = np.asarray(Wk, np.float32).T        # (C, D)
    shared = {
        "wqk4": np.ascontiguousarray(
            np.concatenate([np.tile(wq, (1, 4)), np.tile(wk, (1, 4))], axis=1)
        ).astype(bf),
        "wvT": np.ascontiguousarray(g * np.asarray(Wv, np.float32).T).astype(bf),
        "bqk4": np.ascontiguousarray(np.stack(
            [np.tile(np.asarray(bq, np.float32).reshape(D), 4),
             np.tile(np.asarray(bk, np.float32).reshape(D), 4)], axis=1)),
        "bvb4": np.ascontiguousarray(np.tile(gbv, (1, 4))).astype(bf),
        "identb": np.eye(128, dtype=np.float32).astype(bf),
    }
    return [
        dict(shared, x=np.ascontiguousarray(x[b]), xb=np.ascontiguousarray(x[b]).astype(bf))
        for b in range(B)
    ]


def kernel(x, Wq, bq, Wk, bk, Wv, bv, gamma):
    from concourse.bass_utils import run_bass_kernel_spmd

    nc = get_nc()
    in_maps = make_in_maps(x, Wq, bq, Wk, bk, Wv, bv, gamma)
    res = run_bass_kernel_spmd(nc, in_maps, list(range(NCORES)))
    return np.stack([res.results[b]["y"] for b in range(B)], axis=0)
